# revision 41
# baseline (speedup 1.0000x reference)
"""Trainium2 Bass kernel for the GA block (topk_masking).

Reference semantics (B=128, HW=1024, C=384), pool=1:
    ea   = mean(edge_aggregation, axis=1)            # (B, 1, C)
    ci   = sigmoid(ea)                               # channel importance
    ca   = nodes @ ci                                # (B, HW) node scores
    ni   = sigmoid(ca)
    na   = ni @ nodes                                # (B, C)
    r    = ||cls||_F / ||na||_F   (global over the whole batch)
    cls' = cls + r * na
    out  = concat([cls', nodes sorted ascending by ca, top quarter kept])

Sharding: pure data parallel, 16 batches per core on 8 cores; the global
norms are combined with a tiny AllReduce of squared sums.

Numeric contract: the kept-node ordering must match the reference argsort
of its fp32 scores exactly, so the score computation keeps the verified
structure (pairwise ea fold -> 4 accumulating PE matmuls -> sigmoid;
per-chunk split-half sequential sums of fp32 products).  Scores are
compared in an int32 (x * 2^24) domain.

v3 engine-balanced layout: the cost model serializes DMA transfers on the
issuing queue (SP / ACT-hwdge / Pool-swdge run in parallel), so input
loads are spread across queues and interleaved with compute via a
4-stage software pipeline (load b | ci b-1 | scores b-2 | na b-3):
  SP    nodes DMA;  ACT  ea DMA + sigmoids;  Pool  fold + 6 products +
  na staging;  DVE  2 products + all 8 split-half reduces + misc;
  PE    ci fp32 matmuls + na float32r matmuls (1 cyc/row vs 4).
The topk tail splits each bitonic-merge stage across engines (DVE
compare/max/predicated moves, ACT index copies, Pool mins for L1/L2),
folds bounce on the SP queue, gathers stay on Pool (indirect DMA is
gpsimd-only), node-row writes go on SP.  The norm AllReduce occupies
only the collective cores, overlapping the whole merge tree.
"""

import math
import os
import threading

import numpy as np

import concourse.bass as bass
import concourse.tile as tile
from concourse import bacc, mybir
from concourse.bass_utils import run_bass_kernel_spmd
from concourse.masks import make_identity

F32 = mybir.dt.float32
F32R = mybir.dt.float32r
BF16 = mybir.dt.bfloat16
I32 = mybir.dt.int32
U32 = mybir.dt.uint32
AF = mybir.ActivationFunctionType
OP = mybir.AluOpType

N_CORES = 8
B = 128
HW = 1024
C = 384
NB = B // N_CORES          # batches per core
P = 128
NCH = HW // P              # 8 free-dim chunks of 128 node rows
KEEP = HW // 4             # 256

_PRODP = int(os.environ.get("GA_PRODP", "7"))   # product chunks on Pool
_CASTP = int(os.environ.get("GA_CASTP", "6"))   # bf16-cast chunks on Pool
_NA16 = int(os.environ.get("GA_NA16", "1"))     # 1: bf16 na matmuls
_MINQ = os.environ.get("GA_MINQ", "dve")        # merge min engine (Pool is
                                                # blocked by the collective)
_CPQ = os.environ.get("GA_CPQ", "act")          # stage index-copy engine


def _ap_sub(t_ap, off, pairs):
    """AP with the tile's partition dim plus explicit free-dim pattern."""
    return bass.AP(t_ap.tensor, t_ap.offset + off, [list(t_ap.ap[0])] + pairs)


def _f32r(ap):
    return ap.bitcast(F32R)


def _build_pool1():
    nc = bacc.Bacc(
        "TRN2",
        target_bir_lowering=False,
        debug=False,
        enable_asserts=False,
        num_devices=N_CORES,
    )
    x_h = nc.dram_tensor("x", [NB * (HW + 1), C], F32, kind="ExternalInput")
    cls_h = nc.dram_tensor("cls", [NB, C], F32, kind="ExternalInput")
    ea_h = nc.dram_tensor("ea", [NB * HW, C], F32, kind="ExternalInput")
    out_h = nc.dram_tensor("out", [NB, 1 + KEEP, C], F32, kind="ExternalOutput")

    cc_in = nc.dram_tensor("cc_in", [1, 2], F32)
    cc_out = nc.dram_tensor("cc_out", [1, 2], F32)

    with tile.TileContext(nc) as tc:
        with (
            tc.tile_pool(name="consts", bufs=1) as consts,
            tc.tile_pool(name="eap", bufs=3) as eap,
            tc.tile_pool(name="ndp", bufs=3) as ndp,
            tc.tile_pool(name="work", bufs=3) as work,
            tc.tile_pool(name="workq", bufs=2) as workq,
            tc.tile_pool(name="keep", bufs=1) as keep,
            tc.tile_pool(name="mrg", bufs=1) as mrg,
            tc.tile_pool(name="gath", bufs=8) as gathp,
            tc.tile_pool(name="psum", bufs=2, space="PSUM") as psum,
            tc.tile_pool(name="psumt", bufs=1, space="PSUM") as psumt,
            tc.tile_pool(name="psumn", bufs=1, space="PSUM") as psumn,
        ):
            ones128 = consts.tile([P, P], F32)
            nc.vector.memset(ones128[:], 1.0)
            ident = consts.tile([P, P], F32)
            make_identity(nc, ident[:])
            # row offset of node 0 of local batch b in the flattened x
            rowoff_i = consts.tile([NB, 1], I32)
            nc.gpsimd.iota(
                rowoff_i[:], pattern=[[0, 1]], base=1,
                channel_multiplier=HW + 1,
            )
            rowoff = consts.tile([NB, 1], F32)
            nc.vector.tensor_copy(out=rowoff[:], in_=rowoff_i[:])
            # partition q holds chunk c of batch b with q = 64*c0+32*c1+16*c2+b
            # (c = 4*c2+2*c1+c0), so each merge level folds contiguous
            # partition halves. cvec_f holds c(q) as an f32 per-partition
            # scalar for rebuilding node indices after the per-chunk sort.
            qvec = consts.tile([P, 1], I32)
            nc.gpsimd.iota(
                qvec[:], pattern=[[0, 1]], base=0, channel_multiplier=1,
            )
            cvec = consts.tile([P, 1], I32)
            scr1 = consts.tile([P, 1], I32)
            # c = 4*bit4(q) + 2*bit5(q) + bit6(q)
            nc.vector.tensor_scalar(
                out=cvec[:], in0=qvec[:], scalar1=4, scalar2=1,
                op0=OP.logical_shift_right, op1=OP.bitwise_and,
            )
            nc.vector.tensor_scalar(
                out=cvec[:], in0=cvec[:], scalar1=2, scalar2=None,
                op0=OP.logical_shift_left,
            )
            nc.vector.tensor_scalar(
                out=scr1[:], in0=qvec[:], scalar1=4, scalar2=2,
                op0=OP.logical_shift_right, op1=OP.bitwise_and,
            )
            nc.vector.tensor_tensor(
                out=cvec[:], in0=cvec[:], in1=scr1[:], op=OP.bitwise_or,
            )
            nc.vector.tensor_scalar(
                out=scr1[:], in0=qvec[:], scalar1=6, scalar2=1,
                op0=OP.logical_shift_right, op1=OP.bitwise_and,
            )
            nc.vector.tensor_tensor(
                out=cvec[:], in0=cvec[:], in1=scr1[:], op=OP.bitwise_or,
            )
            cvec_f = consts.tile([P, 1], F32)
            nc.vector.tensor_copy(out=cvec_f[:], in_=cvec[:])
            # partition-0 one-hot rows: segment l of [1, NB*NB] is e_l, used
            # as lhsT to land na rows in psum partition l (lhsT base
            # partition must be 0).
            onehot = consts.tile([1, NB * NB], F32)
            nc.vector.memset(onehot[:], 0.0)
            for l in range(NB):
                nc.vector.memset(onehot[0:1, l * (NB + 1):l * (NB + 1) + 1],
                                 1.0)
            # shift-by-p permutation matrices (shift[k, m] = 1 iff k == m+p):
            # lhsT of the PE "fold" matmuls that move partitions [p, 2p) of a
            # merge tile down to [0, p) without a (slow, sem-laggy) DMA.
            shifts = {}
            for pp in (P // 2, P // 4, P // 8):
                sh = consts.tile([2 * pp, pp], F32, tag=f"sh{pp}")
                rowk_i = consts.tile([2 * pp, 1], I32, tag=f"rki{pp}")
                nc.gpsimd.iota(rowk_i[:], pattern=[[0, 1]], base=0,
                               channel_multiplier=1)
                rowk = consts.tile([2 * pp, 1], F32, tag=f"rk{pp}")
                nc.vector.tensor_copy(out=rowk[:], in_=rowk_i[:])
                colj_i = consts.tile([2 * pp, pp], I32, tag=f"cji{pp}")
                nc.gpsimd.iota(colj_i[:], pattern=[[1, pp]], base=pp,
                               channel_multiplier=0)
                di = consts.tile([2 * pp, pp], F32, tag=f"di{pp}")
                nc.vector.tensor_copy(out=di[:], in_=colj_i[:])
                nc.vector.tensor_scalar(
                    out=di[:], in0=di[:], scalar1=rowk[:, 0:1],
                    scalar2=None, op0=OP.subtract,
                )
                nc.vector.tensor_scalar(
                    out=sh[:], in0=di[:], scalar1=0.0, scalar2=None,
                    op0=OP.is_equal,
                )
                shifts[pp] = sh

            # fp32 node scores, laid out [p, b*8+c]
            ca_allf = keep.tile([P, NB * NCH], F32)
            na_all = keep.tile([NB, C], F32)
            cls_sb = keep.tile([NB, C], F32)
            nc.sync.dma_start(out=cls_sb[:], in_=cls_h[:, :])

            def _emit_ci(b, ea_t, ci_rep):
                fold = work.tile([P, NCH // 2, C], F32, tag="fold")
                # pairwise fold (rows 2j + 2j+1): verified ci rounding draw
                nc.gpsimd.tensor_tensor(
                    out=fold[:], in0=ea_t[:, 0::2, :], in1=ea_t[:, 1::2, :],
                    op=OP.add,
                )
                psum_ci = psum.tile([P, C], F32, tag="psci", space="PSUM")
                for c in range(4):
                    nc.tensor.matmul(
                        out=psum_ci[:], lhsT=ones128[:], rhs=fold[:, c, :],
                        start=(c == 0), stop=(c == 3),
                    )
                nc.scalar.activation(
                    out=ci_rep[:], in_=psum_ci[:], func=AF.Sigmoid,
                    scale=1.0 / HW,
                )

            # batch 15's ea is loaded first (on the otherwise-idle Pool
            # queue) and its ci computed up front, so batch 15's scores
            # only trail its own nodes DMA
            ea15 = keep.tile([P, NCH, C], F32)
            nc.gpsimd.dma_start(
                out=ea15[:],
                in_=ea_h[(NB - 1) * HW:NB * HW, :].rearrange(
                    "(p c) f -> p c f", p=P),
            )
            ci15 = keep.tile([P, C], F32)
            _emit_ci(NB - 1, ea15, ci15[:])

            st = [dict() for _ in range(NB)]

            def stage_load(b):
                nt = ndp.tile([P, NCH, C], F32, tag="nd")
                nc.sync.dma_start(
                    out=nt[:],
                    in_=x_h[
                        b * (HW + 1) + 1:(b + 1) * (HW + 1), :
                    ].rearrange("(p c) f -> p c f", p=P),
                )
                st[b]["nodes"] = nt
                if b != NB - 1:
                    ea_t = eap.tile([P, NCH, C], F32, tag="ea")
                    nc.scalar.dma_start(
                        out=ea_t[:],
                        in_=ea_h[b * HW:(b + 1) * HW, :].rearrange(
                            "(p c) f -> p c f", p=P),
                    )
                    st[b]["ea"] = ea_t

            def stage_ci(j):
                if j == NB - 1:
                    st[j]["ci"] = ci15
                    return
                ci_rep = work.tile([P, C], F32, tag="ci")
                _emit_ci(j, st[j]["ea"], ci_rep[:])
                st[j]["ci"] = ci_rep

            def stage_score(k):
                nodes = st[k]["nodes"]
                ci_rep = st[k]["ci"]
                p_t = workq.tile([P, NCH, C], F32, tag="q")
                for c in range(NCH):
                    eng = nc.gpsimd if c < _PRODP else nc.vector
                    eng.tensor_tensor(
                        out=p_t[:, c, :],
                        in0=nodes[:, c, :],
                        in1=ci_rep[:], op=OP.mult,
                    )

                # per-(chunk, half) sequential fp32 sums: one reduce per
                # chunk collapsing the innermost 192-wide halves (same
                # per-half sequential order as the verified vred variant)
                ca_h = work.tile([P, NCH, 2], F32, tag="cah")
                for c in range(NCH):
                    eng = nc.vector
                    eng.tensor_reduce(
                        out=ca_h[:, c, :],
                        in_=_ap_sub(p_t[:, c, :], 0, [[C // 2, 2], [1, C // 2]]),
                        axis=mybir.AxisListType.X,
                        op=OP.add,
                    )
                ca_f = work.tile([P, NCH], F32, tag="caf")
                nc.vector.tensor_tensor(
                    out=ca_f[:], in0=ca_h[:, :, 0], in1=ca_h[:, :, 1],
                    op=OP.add,
                )
                # column q(b,c) = 64*c0 + 32*c1 + 16*c2 + b (bit-reversed c)
                nc.vector.tensor_copy(
                    out=_ap_sub(ca_allf[:], k, [[64, 2], [32, 2], [16, 2]]),
                    in_=_ap_sub(ca_f[:], 0, [[1, 2], [2, 2], [4, 2]]),
                )
                ni_t = work.tile([P, NCH], BF16 if _NA16 else F32, tag="ni")
                nc.scalar.activation(
                    out=ni_t[:], in_=ca_f[:], func=AF.Sigmoid,
                )
                st[k]["ni"] = ni_t

            # na_all rows are assembled in PSUM by one-hot float32r matmuls
            # (row l of psum_naall accumulates na_stage via lhsT=ident[l,:16])
            # -- avoids 16 lane-crossing Pool DMAs.
            psum_naall = psumn.tile([NB, C], F32, tag="naall", space="PSUM")

            def stage_na(l):
                ni_t = st[l]["ni"]
                nodes = st[l]["n16"] if _NA16 else st[l]["nodes"]
                psum_na = psum.tile([1, C], F32, tag="psna", space="PSUM")
                for c in range(NCH):
                    nc.tensor.matmul(
                        out=psum_na[:], lhsT=ni_t[:, c:c + 1],
                        rhs=nodes[:, c, :],
                        start=(c == 0), stop=(c == NCH - 1),
                    )
                na_stage = work.tile([1, C], F32, tag="nastage")
                nc.vector.tensor_copy(out=na_stage[:], in_=psum_na[:])
                nc.tensor.matmul(
                    out=psum_naall[:],
                    lhsT=onehot[0:1, l * NB:(l + 1) * NB],
                    rhs=na_stage[:],
                    start=(l == 0), stop=(l == NB - 1),
                    skip_group_check=True,
                )

            # 3-deep software pipeline: each engine's per-slot work only
            # depends on results from earlier slots (or earlier ops in the
            # same slot), so the per-engine in-order streams barely stall.
            for slot in range(NB + 2):
                if slot < NB:
                    stage_load(slot)
                if 0 <= slot - 1 < NB:
                    stage_ci(slot - 1)
                if 0 <= slot - 2 < NB:
                    stage_score(slot - 2)
                    stage_na(slot - 2)

            # ---- transpose scores into the per-chunk sort layout.  The
            # sort/merge compares raw f32 scores directly (exact compares;
            # strictly fewer key collisions than the old int32 x*2^24
            # domain, and normal floats survive the PE fold matmuls). ----
            t_ps = psumt.tile([P, P], F32, tag="tx", space="PSUM")
            nc.tensor.transpose(out=t_ps[:], in_=ca_allf[:], identity=ident[:])
            catf = keep.tile([P, P], F32)
            nc.scalar.copy(out=catf[:], in_=t_ps[:])

            # ---- global norm ratio: partial squared sums + AllReduce.
            # The collective blocks the Pool engine for its full modeled
            # duration, so it is launched as early as possible and Pool is
            # kept otherwise idle until the (post-collective) gathers. ----
            nc.vector.tensor_copy(out=na_all[:], in_=psum_naall[:])
            sq2 = keep.tile([NB, 2], F32)
            sq_scr = keep.tile([NB, C], F32)
            nc.scalar.activation(
                out=sq_scr[:], in_=cls_sb[:], func=AF.Square,
                accum_out=sq2[:, 0:1],
            )
            nc.scalar.activation(
                out=sq_scr[:], in_=na_all[:], func=AF.Square,
                accum_out=sq2[:, 1:2],
            )
            part_ps = psumt.tile([1, 2], F32, tag="cc2", space="PSUM")
            nc.tensor.matmul(
                out=part_ps[:], lhsT=ones128[:NB, 0:1], rhs=sq2[:],
                start=True, stop=True,
            )
            part_sb = keep.tile([1, 2], F32)
            nc.scalar.copy(out=part_sb[:], in_=part_ps[:])
            nc.sync.dma_start(out=cc_in[:], in_=part_sb[:])
            nc.gpsimd.collective_compute(
                "AllReduce",
                OP.add,
                replica_groups=[list(range(N_CORES))],
                ins=[cc_in[:].opt()],
                outs=[cc_out[:].opt()],
            )
            sums_sb = keep.tile([1, 2], F32)
            nc.gpsimd.dma_start(out=sums_sb[:], in_=cc_out[:])

            _topk_h2(nc, keep, mrg, gathp, psumt, catf, cvec_f, rowoff,
                     ident, shifts, x_h, out_h)

            # ---- post-collective cls row (engine streams reach these ops
            # only after the merge work, by which time cc_out is ready) ----
            rep_ps = psumt.tile([NB, 2], F32, tag="cc2", space="PSUM")
            nc.tensor.matmul(
                out=rep_ps[:], lhsT=ones128[0:1, :NB], rhs=sums_sb[:],
                start=True, stop=True,
            )
            rep_sb = keep.tile([NB, 2], F32)
            nc.scalar.copy(out=rep_sb[:], in_=rep_ps[:])
            inv_na = keep.tile([NB, 1], F32)
            nc.vector.reciprocal(out=inv_na[:], in_=rep_sb[:, 1:2])
            ratio = keep.tile([NB, 1], F32)
            nc.vector.tensor_tensor(
                out=ratio[:], in0=rep_sb[:, 0:1], in1=inv_na[:], op=OP.mult
            )
            r_sb = keep.tile([NB, 1], F32)
            nc.scalar.activation(out=r_sb[:], in_=ratio[:], func=AF.Sqrt)

            cls_out = keep.tile([NB, C], F32)
            nc.vector.scalar_tensor_tensor(
                out=cls_out[:], in0=na_all[:], scalar=r_sb[:, 0:1],
                in1=cls_sb[:], op0=OP.mult, op1=OP.add,
            )
            nc.sync.dma_start(out=out_h[:, 0, :], in_=cls_out[:])

    nc.compile()
    return nc


def _topk_h2(nc, keep, mrg, gathp, psumt, catf, cvec_f, rowoff, ident,
             shifts, x_h, out_h):
    """Hierarchical top-256: per-chunk sort on 128 partitions, then a
    bitonic merge tree with (f32) index tracking.  Stage ops are split
    across DVE (compare / max / min / predicated moves) and ACT (index
    copies); partition-crossing folds are PE shift-matmuls (values and
    indices are normal f32, exact under x*1 + 0 accumulation)."""
    # ---- per-chunk descending sort (values + in-chunk positions) ----
    v0 = keep.tile([P, P], F32)
    i0u = keep.tile([P, P], U32)
    for k in range(P // 8):
        sl = slice(8 * k, 8 * k + 8)
        nc.vector.max(out=v0[:, sl], in_=catf[:])
        nc.vector.max_index(
            out=i0u[:, sl], in_max=v0[:, sl], in_values=catf[:]
        )
        if k != P // 8 - 1:
            nc.vector.match_replace(
                out=catf[:], in_to_replace=v0[:, sl], in_values=catf[:],
                imm_value=-1.6e9,
            )
    # global node index n = pos*8 + c (as exact f32; n = p*8 + c)
    i0f = keep.tile([P, P], F32)
    nc.vector.tensor_copy(out=i0f[:], in_=i0u[:])
    n0 = keep.tile([P, P], F32)
    nc.vector.tensor_scalar(
        out=n0[:], in0=i0f[:], scalar1=8.0, scalar2=None, op0=OP.mult,
    )
    nc.vector.tensor_scalar(
        out=n0[:], in0=n0[:], scalar1=cvec_f[:, 0:1], scalar2=None,
        op0=OP.add,
    )

    cp_eng = nc.scalar if _CPQ == "act" else nc.vector
    min_eng = nc.gpsimd if _MINQ == "pool" else nc.vector

    def fold(dst, src, parts, width, dt, src_cols=None, ptag="fold"):
        """[2p, w] -> [p, 2w]: partitions m and m+p land in row m.
        Lower half is an ACT copy; the partition-crossing upper half is a
        PE shift-matmul (out[m,:] = src[m+p,:]) -- ~1us latency vs the
        multi-us DMA+semaphore path."""
        sw = width if src_cols is None else src_cols
        cp_eng.copy(out=dst[0:parts, 0:width], in_=src[0:parts, 0:sw])
        ps = psumt.tile([parts, width], F32, tag=ptag, space="PSUM")
        nc.tensor.matmul(
            out=ps[:], lhsT=shifts[parts][:], rhs=src[0:2 * parts, 0:sw],
            start=True, stop=True,
        )
        cp_eng.copy(out=dst[0:parts, width:2 * width], in_=ps[:])

    def stage(vin, vout, iin, iout, mask, width, d, split=False,
              top_only=False, deep_min=False, ovr=None):
        """One compare-exchange stage on [*, width] tiles (descending)."""
        if split:
            n = width // 2
            if ovr is not None:
                va, vb, ia, ib = ovr
            else:
                va = _ap_sub(vin, 0, [[1, n]])
                vb = _ap_sub(vin, width - 1, [[-1, n]])
                ia = _ap_sub(iin, 0, [[1, n]])
                ib = _ap_sub(iin, width - 1, [[-1, n]])
            vlo = _ap_sub(vout, 0, [[1, n]])
            vhi = _ap_sub(vout, n, [[1, n]])
            ilo = _ap_sub(iout, 0, [[1, n]])
            ihi = _ap_sub(iout, n, [[1, n]])
            m = _ap_sub(mask, 0, [[1, n]])
        else:
            nb = width // (2 * d)
            va = _ap_sub(vin, 0, [[2 * d, nb], [1, d]])
            vb = _ap_sub(vin, d, [[2 * d, nb], [1, d]])
            ia = _ap_sub(iin, 0, [[2 * d, nb], [1, d]])
            ib = _ap_sub(iin, d, [[2 * d, nb], [1, d]])
            vlo = _ap_sub(vout, 0, [[2 * d, nb], [1, d]])
            vhi = _ap_sub(vout, d, [[2 * d, nb], [1, d]])
            ilo = _ap_sub(iout, 0, [[2 * d, nb], [1, d]])
            ihi = _ap_sub(iout, d, [[2 * d, nb], [1, d]])
            m = _ap_sub(mask, 0, [[2 * d, nb], [1, d]])
        # reversed-stride APs (split stages) stay on DVE -- the validated
        # path for negative strides
        cp = nc.vector.tensor_copy if split else (
            lambda out, in_: cp_eng.copy(out=out, in_=in_))
        nc.vector.tensor_tensor(out=m, in0=va, in1=vb, op=OP.is_gt)
        nc.vector.tensor_tensor(out=vlo, in0=va, in1=vb, op=OP.max)
        cp(out=ilo, in_=ib)
        nc.vector.copy_predicated(out=ilo, mask=m, data=ia)
        if not top_only:
            vmin = nc.vector if deep_min else min_eng
            vmin.tensor_tensor(out=vhi, in0=va, in1=vb, op=OP.min)
            cp(out=ihi, in_=ia)
            nc.vector.copy_predicated(out=ihi, mask=m, data=ib)

    # ---- L1: merge chunk pairs -> sorted 256 per (b, pair) ----
    va1 = mrg.tile([P // 2, 2 * P], F32)
    vb1 = mrg.tile([P // 2, 2 * P], F32)
    ia1 = mrg.tile([P // 2, 2 * P], F32)
    ib1 = mrg.tile([P // 2, 2 * P], F32)
    m1 = mrg.tile([P // 2, 2 * P], I32)
    fold(va1, v0, P // 2, P, F32)
    fold(ia1, n0, P // 2, P, F32, ptag="tx")
    cur_v, cur_i, alt_v, alt_i = va1, ia1, vb1, ib1
    stage(cur_v[:], alt_v[:], cur_i[:], alt_i[:], m1[:], 2 * P, P, split=True)
    cur_v, alt_v = alt_v, cur_v
    cur_i, alt_i = alt_i, cur_i
    d = P // 2
    while d >= 1:
        stage(cur_v[:], alt_v[:], cur_i[:], alt_i[:], m1[:], 2 * P, d)
        cur_v, alt_v = alt_v, cur_v
        cur_i, alt_i = alt_i, cur_i
        d //= 2

    def merge_levels(cv, ci_, av, ai, mm, keepw, deep_min=False,
                     half_only=False):
        d = keepw // 2
        first = True
        h = keepw // 2
        while d >= 1:
            if half_only and not first:
                stage(cv[:, h:], av[:, h:], ci_[:, h:], ai[:, h:], mm[:],
                      h, d, deep_min=deep_min)
            else:
                stage(cv[:], av[:], ci_[:], ai[:], mm[:], keepw, d,
                      deep_min=deep_min)
            cv, av = av, cv
            ci_, ai = ai, ci_
            d //= 2
            first = False
        return cv, ci_

    def merge_top(v_in, i_in, parts, keepw, mrgp, tagp, deep_min=False,
                  half_only=False):
        """[2*parts, keepw-lists] -> [parts, keepw] sorted top keepw.
        With half_only, only ranks [keepw/2, keepw) come out sorted (the
        top block stays bitonic) -- the final levels run on half width."""
        vs = mrgp.tile([parts, 2 * keepw], F32, tag=f"vs{tagp}")
        is_ = mrgp.tile([parts, 2 * keepw], F32, tag=f"is{tagp}")
        fold(vs, v_in, parts, keepw, F32)
        fold(is_, i_in, parts, keepw, F32, ptag="tx")
        vA = mrgp.tile([parts, keepw], F32, tag=f"vA{tagp}")
        iA = mrgp.tile([parts, keepw], F32, tag=f"iA{tagp}")
        vB = mrgp.tile([parts, keepw], F32, tag=f"vB{tagp}")
        iB = mrgp.tile([parts, keepw], F32, tag=f"iB{tagp}")
        mm = mrgp.tile([parts, 2 * keepw], I32, tag=f"mm{tagp}")
        # split stage keeping only the top half
        stage(vs[:], vA[:], is_[:], iA[:], mm[:], 2 * keepw, keepw,
              split=True, top_only=True)
        cv, ci_ = merge_levels(vA, iA, vB, iB, mm, keepw, deep_min=deep_min,
                               half_only=half_only)
        return cv, ci_, vs, is_

    # ---- L2: [64,256]x2 -> [32, 256] top halves ----
    v2, i2, _, _ = merge_top(cur_v, cur_i, P // 4, 2 * P, mrg, "a")

    def emit_out_half(idx_rev_src, lo):
        """Gather + write out rows [1+lo, 1+lo+128) from reversed-rank
        node-index AP idx_rev_src ([16, 128] f32, ascending out order)."""
        rows = keep.tile([NB, P], F32, tag=f"rows{lo}")
        nc.vector.tensor_copy(out=rows[:], in_=idx_rev_src)
        nc.vector.tensor_scalar(
            out=rows[:], in0=rows[:], scalar1=rowoff[:, 0:1], scalar2=None,
            op0=OP.add,
        )
        r_ps = psumt.tile([P, NB], F32, tag="tx", space="PSUM")
        nc.tensor.transpose(out=r_ps[:], in_=rows[:], identity=ident[:NB, :NB])
        rowst = keep.tile([P, NB], I32, tag=f"rowst{lo}")
        nc.vector.tensor_copy(out=rowst[:], in_=r_ps[:])
        for bb in range(NB):
            g = gathp.tile([P, C], F32, tag="g")
            nc.gpsimd.indirect_dma_start(
                out=g[:], out_offset=None, in_=x_h[:, :],
                in_offset=bass.IndirectOffsetOnAxis(
                    ap=rowst[:, bb:bb + 1], axis=0),
            )
            nc.sync.dma_start(out=out_h[bb, 1 + lo:1 + lo + P, :], in_=g[:])

    # ---- L3 folds: [32, 256]x2 -> [16, 512] value/index planes.  Both
    # the quick top-128 (Lq, prefixes at cols [0:128) and [256:384)) and
    # the full L3 merge read these. ----
    vs3 = mrg.tile([NB, 4 * P], F32, tag="vsb")
    is3 = mrg.tile([NB, 4 * P], F32, tag="isb")
    fold(vs3, v2, NB, 2 * P, F32)
    fold(is3, i2, NB, 2 * P, F32, ptag="tx")

    # ---- quick top-128 (ranks 0..127) from the two top-128 prefixes,
    # so the h=1 gathers overlap the full L3 merge ----
    vqA = mrg.tile([NB, P], F32, tag="vAq")
    iqA = mrg.tile([NB, P], F32, tag="iAq")
    vqB = mrg.tile([NB, P], F32, tag="vBq")
    iqB = mrg.tile([NB, P], F32, tag="iBq")
    mq = mrg.tile([NB, 2 * P], I32, tag="mmq")
    stage(None, vqA[:], None, iqA[:], mq[:], 2 * P, P, split=True,
          top_only=True,
          ovr=(_ap_sub(vs3[:], 0, [[1, P]]),
               _ap_sub(vs3[:], 2 * P + P - 1, [[-1, P]]),
               _ap_sub(is3[:], 0, [[1, P]]),
               _ap_sub(is3[:], 2 * P + P - 1, [[-1, P]])))
    vq, iq = merge_levels(vqA, iqA, vqB, iqB, mq, P, deep_min=True)
    # ranks 0..127: out rows 129..256 ascending = rank 127-t
    emit_out_half(iq[:, P - 1::-1], P)

    # ---- L3: -> [16, 256] global descending top-256 + node indices.
    # Only ranks 128..255 are consumed (h1 comes from Lq), so the final
    # levels sort just that block (half_only). ----
    vA3 = mrg.tile([NB, 2 * P], F32, tag="vAb")
    iA3 = mrg.tile([NB, 2 * P], F32, tag="iAb")
    vB3 = mrg.tile([NB, 2 * P], F32, tag="vBb")
    iB3 = mrg.tile([NB, 2 * P], F32, tag="iBb")
    m3 = mrg.tile([NB, 4 * P], I32, tag="mmb")
    stage(vs3[:], vA3[:], is3[:], iA3[:], m3[:], 4 * P, 2 * P, split=True,
          top_only=True)
    v3, i3 = merge_levels(vA3, iA3, vB3, iB3, m3, 2 * P, deep_min=True,
                          half_only=True)
    # ranks 128..255: out rows 1..128 ascending = rank 255-j
    emit_out_half(i3[:, 2 * P - 1:P - 1:-1], 0)


_CACHE = {}
_LOCK = threading.Lock()


def _get_program(pool):
    with _LOCK:
        if pool not in _CACHE:
            if pool:
                _CACHE[pool] = _build_pool1()
            else:
                raise NotImplementedError("pool=0 path not implemented")
        return _CACHE[pool]


def kernel(x, cls_token, edge_aggregation, pool):
    x = np.ascontiguousarray(np.asarray(x, dtype=np.float32))
    cls_token = np.ascontiguousarray(np.asarray(cls_token, dtype=np.float32))
    ea = np.ascontiguousarray(np.asarray(edge_aggregation, dtype=np.float32))
    pool_i = int(np.asarray(pool))

    nc = _get_program(bool(pool_i))

    in_maps = []
    for core in range(N_CORES):
        s = slice(core * NB, (core + 1) * NB)
        in_maps.append({
            "x": x[s].reshape(NB * (HW + 1), C),
            "cls": cls_token[s].reshape(NB, C),
            "ea": ea[s].reshape(NB * HW, C),
        })
    kw = {}
    if os.environ.get("GA_TRACE"):
        kw = {"trace": True}
    res = run_bass_kernel_spmd(nc, in_maps, core_ids=list(range(N_CORES)), **kw)
    global _LAST_RESULTS, _LAST_EXEC_NS
    _LAST_RESULTS = res.results
    _LAST_EXEC_NS = res.exec_time_ns
    out = np.concatenate([res.results[c]["out"] for c in range(N_CORES)], axis=0)
    return out.reshape(B, 1 + KEEP, C)


_LAST_RESULTS = None
_LAST_EXEC_NS = None


# revision 48
# speedup vs baseline: 1.0065x; 1.0065x over previous
"""Trainium2 Bass kernel for the GA block (topk_masking).

Reference semantics (B=128, HW=1024, C=384), pool=1:
    ea   = mean(edge_aggregation, axis=1)            # (B, 1, C)
    ci   = sigmoid(ea)                               # channel importance
    ca   = nodes @ ci                                # (B, HW) node scores
    ni   = sigmoid(ca)
    na   = ni @ nodes                                # (B, C)
    r    = ||cls||_F / ||na||_F   (global over the whole batch)
    cls' = cls + r * na
    out  = concat([cls', nodes sorted ascending by ca, top quarter kept])

Sharding: pure data parallel, 16 batches per core on 8 cores; the global
norms are combined with a tiny AllReduce of squared sums.

Numeric contract: the kept-node ordering must match the reference argsort
of its fp32 scores exactly, so the score computation keeps the verified
structure (pairwise ea fold -> 4 accumulating PE matmuls -> sigmoid;
per-chunk split-half sequential sums of fp32 products).  Scores are
compared in an int32 (x * 2^24) domain.

v3 engine-balanced layout: the cost model serializes DMA transfers on the
issuing queue (SP / ACT-hwdge / Pool-swdge run in parallel), so input
loads are spread across queues and interleaved with compute via a
4-stage software pipeline (load b | ci b-1 | scores b-2 | na b-3):
  SP    nodes DMA;  ACT  ea DMA + sigmoids;  Pool  fold + 6 products +
  na staging;  DVE  2 products + all 8 split-half reduces + misc;
  PE    ci fp32 matmuls + na float32r matmuls (1 cyc/row vs 4).
The topk tail splits each bitonic-merge stage across engines (DVE
compare/max/predicated moves, ACT index copies, Pool mins for L1/L2),
folds bounce on the SP queue, gathers stay on Pool (indirect DMA is
gpsimd-only), node-row writes go on SP.  The norm AllReduce occupies
only the collective cores, overlapping the whole merge tree.
"""

import math
import os
import threading

import numpy as np

import concourse.bass as bass
import concourse.tile as tile
from concourse import bacc, mybir
from concourse.bass_utils import run_bass_kernel_spmd
from concourse.masks import make_identity

F32 = mybir.dt.float32
F32R = mybir.dt.float32r
BF16 = mybir.dt.bfloat16
I32 = mybir.dt.int32
U32 = mybir.dt.uint32
AF = mybir.ActivationFunctionType
OP = mybir.AluOpType

N_CORES = 8
B = 128
HW = 1024
C = 384
NB = B // N_CORES          # batches per core
P = 128
NCH = HW // P              # 8 free-dim chunks of 128 node rows
KEEP = HW // 4             # 256

_PRODP = int(os.environ.get("GA_PRODP", "7"))   # product chunks on Pool
_CASTP = int(os.environ.get("GA_CASTP", "6"))   # bf16-cast chunks on Pool
_NA16 = int(os.environ.get("GA_NA16", "1"))     # 1: bf16 na matmuls
_MINQ = os.environ.get("GA_MINQ", "dve")        # merge min engine (Pool is
                                                # blocked by the collective)
_CPQ = os.environ.get("GA_CPQ", "act")          # stage index-copy engine


def _ap_sub(t_ap, off, pairs):
    """AP with the tile's partition dim plus explicit free-dim pattern."""
    return bass.AP(t_ap.tensor, t_ap.offset + off, [list(t_ap.ap[0])] + pairs)


def _f32r(ap):
    return ap.bitcast(F32R)


def _build_pool1():
    nc = bacc.Bacc(
        "TRN2",
        target_bir_lowering=False,
        debug=False,
        enable_asserts=False,
        num_devices=N_CORES,
    )
    x_h = nc.dram_tensor("x", [NB * (HW + 1), C], F32, kind="ExternalInput")
    cls_h = nc.dram_tensor("cls", [NB, C], F32, kind="ExternalInput")
    ea_h = nc.dram_tensor("ea", [NB * HW, C], F32, kind="ExternalInput")
    out_h = nc.dram_tensor("out", [NB, 1 + KEEP, C], F32, kind="ExternalOutput")

    cc_in = nc.dram_tensor("cc_in", [1, 2], F32)
    cc_out = nc.dram_tensor("cc_out", [N_CORES, 2], F32)

    with tile.TileContext(nc) as tc:
        with (
            tc.tile_pool(name="consts", bufs=1) as consts,
            tc.tile_pool(name="eap", bufs=3) as eap,
            tc.tile_pool(name="ndp", bufs=3) as ndp,
            tc.tile_pool(name="work", bufs=3) as work,
            tc.tile_pool(name="workq", bufs=2) as workq,
            tc.tile_pool(name="keep", bufs=1) as keep,
            tc.tile_pool(name="mrg", bufs=1) as mrg,
            tc.tile_pool(name="gath", bufs=8) as gathp,
            tc.tile_pool(name="psum", bufs=2, space="PSUM") as psum,
            tc.tile_pool(name="psumt", bufs=1, space="PSUM") as psumt,
            tc.tile_pool(name="psumn", bufs=1, space="PSUM") as psumn,
        ):
            ones128 = consts.tile([P, P], F32)
            nc.vector.memset(ones128[:], 1.0)
            ident = consts.tile([P, P], F32)
            make_identity(nc, ident[:])
            # row offset of node 0 of local batch b in the flattened x
            rowoff_i = consts.tile([NB, 1], I32)
            nc.gpsimd.iota(
                rowoff_i[:], pattern=[[0, 1]], base=1,
                channel_multiplier=HW + 1,
            )
            rowoff = consts.tile([NB, 1], F32)
            nc.vector.tensor_copy(out=rowoff[:], in_=rowoff_i[:])
            # partition q holds chunk c of batch b with q = 64*c0+32*c1+16*c2+b
            # (c = 4*c2+2*c1+c0), so each merge level folds contiguous
            # partition halves. cvec_f holds c(q) as an f32 per-partition
            # scalar for rebuilding node indices after the per-chunk sort.
            qvec = consts.tile([P, 1], I32)
            nc.gpsimd.iota(
                qvec[:], pattern=[[0, 1]], base=0, channel_multiplier=1,
            )
            cvec = consts.tile([P, 1], I32)
            scr1 = consts.tile([P, 1], I32)
            # c = 4*bit4(q) + 2*bit5(q) + bit6(q)
            nc.vector.tensor_scalar(
                out=cvec[:], in0=qvec[:], scalar1=4, scalar2=1,
                op0=OP.logical_shift_right, op1=OP.bitwise_and,
            )
            nc.vector.tensor_scalar(
                out=cvec[:], in0=cvec[:], scalar1=2, scalar2=None,
                op0=OP.logical_shift_left,
            )
            nc.vector.tensor_scalar(
                out=scr1[:], in0=qvec[:], scalar1=4, scalar2=2,
                op0=OP.logical_shift_right, op1=OP.bitwise_and,
            )
            nc.vector.tensor_tensor(
                out=cvec[:], in0=cvec[:], in1=scr1[:], op=OP.bitwise_or,
            )
            nc.vector.tensor_scalar(
                out=scr1[:], in0=qvec[:], scalar1=6, scalar2=1,
                op0=OP.logical_shift_right, op1=OP.bitwise_and,
            )
            nc.vector.tensor_tensor(
                out=cvec[:], in0=cvec[:], in1=scr1[:], op=OP.bitwise_or,
            )
            cvec_f = consts.tile([P, 1], F32)
            nc.vector.tensor_copy(out=cvec_f[:], in_=cvec[:])
            # partition-0 one-hot rows: segment l of [1, NB*NB] is e_l, used
            # as lhsT to land na rows in psum partition l (lhsT base
            # partition must be 0).
            onehot = consts.tile([1, NB * NB], F32)
            nc.vector.memset(onehot[:], 0.0)
            for l in range(NB):
                nc.vector.memset(onehot[0:1, l * (NB + 1):l * (NB + 1) + 1],
                                 1.0)
            # shift-by-p permutation matrices (shift[k, m] = 1 iff k == m+p):
            # lhsT of the PE "fold" matmuls that move partitions [p, 2p) of a
            # merge tile down to [0, p) without a (slow, sem-laggy) DMA.
            shifts = {}
            for pp in (P // 2, P // 4, P // 8):
                sh = consts.tile([2 * pp, pp], F32, tag=f"sh{pp}")
                rowk_i = consts.tile([2 * pp, 1], I32, tag=f"rki{pp}")
                nc.gpsimd.iota(rowk_i[:], pattern=[[0, 1]], base=0,
                               channel_multiplier=1)
                rowk = consts.tile([2 * pp, 1], F32, tag=f"rk{pp}")
                nc.vector.tensor_copy(out=rowk[:], in_=rowk_i[:])
                colj_i = consts.tile([2 * pp, pp], I32, tag=f"cji{pp}")
                nc.gpsimd.iota(colj_i[:], pattern=[[1, pp]], base=pp,
                               channel_multiplier=0)
                di = consts.tile([2 * pp, pp], F32, tag=f"di{pp}")
                nc.vector.tensor_copy(out=di[:], in_=colj_i[:])
                nc.vector.tensor_scalar(
                    out=di[:], in0=di[:], scalar1=rowk[:, 0:1],
                    scalar2=None, op0=OP.subtract,
                )
                nc.vector.tensor_scalar(
                    out=sh[:], in0=di[:], scalar1=0.0, scalar2=None,
                    op0=OP.is_equal,
                )
                shifts[pp] = sh

            # fp32 node scores, laid out [p, b*8+c]
            ca_allf = keep.tile([P, NB * NCH], F32)
            na_all = keep.tile([NB, C], F32)
            cls_sb = keep.tile([NB, C], F32)
            nc.sync.dma_start(out=cls_sb[:], in_=cls_h[:, :])

            def _emit_ci(b, ea_t, ci_rep):
                fold = work.tile([P, NCH // 2, C], F32, tag="fold")
                # pairwise fold (rows 2j + 2j+1): verified ci rounding draw
                nc.gpsimd.tensor_tensor(
                    out=fold[:], in0=ea_t[:, 0::2, :], in1=ea_t[:, 1::2, :],
                    op=OP.add,
                )
                psum_ci = psum.tile([P, C], F32, tag="psci", space="PSUM")
                for c in range(4):
                    nc.tensor.matmul(
                        out=psum_ci[:], lhsT=ones128[:], rhs=fold[:, c, :],
                        start=(c == 0), stop=(c == 3),
                    )
                nc.scalar.activation(
                    out=ci_rep[:], in_=psum_ci[:], func=AF.Sigmoid,
                    scale=1.0 / HW,
                )

            # batch 15's ea is loaded first (on the otherwise-idle Pool
            # queue) and its ci computed up front, so batch 15's scores
            # only trail its own nodes DMA
            ea15 = keep.tile([P, NCH, C], F32)
            nc.gpsimd.dma_start(
                out=ea15[:],
                in_=ea_h[(NB - 1) * HW:NB * HW, :].rearrange(
                    "(p c) f -> p c f", p=P),
            )
            ci15 = keep.tile([P, C], F32)
            _emit_ci(NB - 1, ea15, ci15[:])

            st = [dict() for _ in range(NB)]

            def stage_load(b):
                nt = ndp.tile([P, NCH, C], F32, tag="nd")
                nc.sync.dma_start(
                    out=nt[:],
                    in_=x_h[
                        b * (HW + 1) + 1:(b + 1) * (HW + 1), :
                    ].rearrange("(p c) f -> p c f", p=P),
                )
                st[b]["nodes"] = nt
                if b != NB - 1:
                    ea_t = eap.tile([P, NCH, C], F32, tag="ea")
                    nc.scalar.dma_start(
                        out=ea_t[:],
                        in_=ea_h[b * HW:(b + 1) * HW, :].rearrange(
                            "(p c) f -> p c f", p=P),
                    )
                    st[b]["ea"] = ea_t

            def stage_ci(j):
                if j == NB - 1:
                    st[j]["ci"] = ci15
                    return
                ci_rep = work.tile([P, C], F32, tag="ci")
                _emit_ci(j, st[j]["ea"], ci_rep[:])
                st[j]["ci"] = ci_rep

            def stage_score(k):
                nodes = st[k]["nodes"]
                ci_rep = st[k]["ci"]
                p_t = workq.tile([P, NCH, C], F32, tag="q")
                for c in range(NCH):
                    eng = nc.gpsimd if c < _PRODP else nc.vector
                    eng.tensor_tensor(
                        out=p_t[:, c, :],
                        in0=nodes[:, c, :],
                        in1=ci_rep[:], op=OP.mult,
                    )

                # per-(chunk, half) sequential fp32 sums: one reduce per
                # chunk collapsing the innermost 192-wide halves (same
                # per-half sequential order as the verified vred variant)
                ca_h = work.tile([P, NCH, 2], F32, tag="cah")
                for c in range(NCH):
                    eng = nc.vector
                    eng.tensor_reduce(
                        out=ca_h[:, c, :],
                        in_=_ap_sub(p_t[:, c, :], 0, [[C // 2, 2], [1, C // 2]]),
                        axis=mybir.AxisListType.X,
                        op=OP.add,
                    )
                ca_f = work.tile([P, NCH], F32, tag="caf")
                nc.vector.tensor_tensor(
                    out=ca_f[:], in0=ca_h[:, :, 0], in1=ca_h[:, :, 1],
                    op=OP.add,
                )
                # column q(b,c) = 64*c0 + 32*c1 + 16*c2 + b (bit-reversed c)
                nc.vector.tensor_copy(
                    out=_ap_sub(ca_allf[:], k, [[64, 2], [32, 2], [16, 2]]),
                    in_=_ap_sub(ca_f[:], 0, [[1, 2], [2, 2], [4, 2]]),
                )
                ni_t = work.tile([P, NCH], BF16 if _NA16 else F32, tag="ni")
                nc.scalar.activation(
                    out=ni_t[:], in_=ca_f[:], func=AF.Sigmoid,
                )
                st[k]["ni"] = ni_t

            # na_all rows are assembled in PSUM by one-hot float32r matmuls
            # (row l of psum_naall accumulates na_stage via lhsT=ident[l,:16])
            # -- avoids 16 lane-crossing Pool DMAs.
            psum_naall = psumn.tile([NB, C], F32, tag="naall", space="PSUM")

            def stage_na(l):
                ni_t = st[l]["ni"]
                if _NA16:
                    # bf16 shadow of the nodes for the (loose-tolerance) na
                    # matmuls, split across Pool and DVE.  Emitted after the
                    # score stage so the last batches' (topk-critical) score
                    # columns aren't queued behind the casts.
                    nodes = workq.tile([P, NCH, C], BF16, tag="n16")
                    if _CASTP:
                        nc.gpsimd.tensor_copy(out=nodes[:, :_CASTP, :],
                                              in_=st[l]["nodes"][:, :_CASTP, :])
                    if _CASTP < NCH:
                        nc.vector.tensor_copy(out=nodes[:, _CASTP:, :],
                                              in_=st[l]["nodes"][:, _CASTP:, :])
                else:
                    nodes = st[l]["nodes"]
                psum_na = psum.tile([1, C], F32, tag="psna", space="PSUM")
                for c in range(NCH):
                    nc.tensor.matmul(
                        out=psum_na[:], lhsT=ni_t[:, c:c + 1],
                        rhs=nodes[:, c, :],
                        start=(c == 0), stop=(c == NCH - 1),
                    )
                na_stage = work.tile([1, C], F32, tag="nastage")
                nc.vector.tensor_copy(out=na_stage[:], in_=psum_na[:])
                nc.tensor.matmul(
                    out=psum_naall[:],
                    lhsT=onehot[0:1, l * NB:(l + 1) * NB],
                    rhs=na_stage[:],
                    start=(l == 0), stop=(l == NB - 1),
                    skip_group_check=True,
                )

            # 3-deep software pipeline: each engine's per-slot work only
            # depends on results from earlier slots (or earlier ops in the
            # same slot), so the per-engine in-order streams barely stall.
            for slot in range(NB + 2):
                if slot < NB:
                    stage_load(slot)
                if 0 <= slot - 1 < NB:
                    stage_ci(slot - 1)
                if 0 <= slot - 2 < NB:
                    stage_score(slot - 2)
                    stage_na(slot - 2)

            # ---- transpose scores into the per-chunk sort layout.  The
            # sort/merge compares raw f32 scores directly (exact compares;
            # strictly fewer key collisions than the old int32 x*2^24
            # domain, and normal floats survive the PE fold matmuls). ----
            t_ps = psumt.tile([P, P], F32, tag="tx", space="PSUM")
            nc.tensor.transpose(out=t_ps[:], in_=ca_allf[:], identity=ident[:])
            catf = keep.tile([P, P], F32)
            nc.scalar.copy(out=catf[:], in_=t_ps[:])

            # ---- global norm ratio: partial squared sums + AllReduce.
            # The collective blocks the Pool engine for its full modeled
            # duration, so it is launched as early as possible and Pool is
            # kept otherwise idle until the (post-collective) gathers. ----
            nc.vector.tensor_copy(out=na_all[:], in_=psum_naall[:])
            sq2 = keep.tile([NB, 2], F32)
            sq_scr = keep.tile([NB, C], F32)
            nc.scalar.activation(
                out=sq_scr[:], in_=cls_sb[:], func=AF.Square,
                accum_out=sq2[:, 0:1],
            )
            nc.scalar.activation(
                out=sq_scr[:], in_=na_all[:], func=AF.Square,
                accum_out=sq2[:, 1:2],
            )
            part_ps = psumt.tile([1, 2], F32, tag="cc2", space="PSUM")
            nc.tensor.matmul(
                out=part_ps[:], lhsT=ones128[:NB, 0:1], rhs=sq2[:],
                start=True, stop=True,
            )
            part_sb = keep.tile([1, 2], F32)
            nc.scalar.copy(out=part_sb[:], in_=part_ps[:])
            nc.sync.dma_start(out=cc_in[:], in_=part_sb[:])
            # AllGather + local sum instead of AllReduce: the collective
            # blocks the Pool engine for its modeled duration, and the cost
            # model prices AllGather at ~15us vs AllReduce's ~28us.
            nc.gpsimd.collective_compute(
                "AllGather",
                OP.bypass,
                replica_groups=[list(range(N_CORES))],
                ins=[cc_in[:].opt()],
                outs=[cc_out[:, :].opt()],
            )
            parts8 = keep.tile([1, 2, N_CORES], F32)
            nc.gpsimd.dma_start(
                out=parts8[:],
                in_=_ap_sub(cc_out[0:1, 0:1], 0, [[1, 2], [2, N_CORES]]))

            _topk_h2(nc, keep, mrg, gathp, psumt, catf, cvec_f, rowoff,
                     ident, shifts, x_h, out_h)

            # ---- post-collective cls row (emitted after the merge so no
            # engine stream stalls on the collective mid-topk) ----
            sums_sb = keep.tile([1, 2], F32)
            nc.vector.tensor_reduce(
                out=sums_sb[:], in_=parts8[:],
                axis=mybir.AxisListType.X, op=OP.add,
            )
            rep_ps = psumt.tile([NB, 2], F32, tag="cc2", space="PSUM")
            nc.tensor.matmul(
                out=rep_ps[:], lhsT=ones128[0:1, :NB], rhs=sums_sb[:],
                start=True, stop=True,
            )
            rep_sb = keep.tile([NB, 2], F32)
            nc.scalar.copy(out=rep_sb[:], in_=rep_ps[:])
            inv_na = keep.tile([NB, 1], F32)
            nc.vector.reciprocal(out=inv_na[:], in_=rep_sb[:, 1:2])
            ratio = keep.tile([NB, 1], F32)
            nc.vector.tensor_tensor(
                out=ratio[:], in0=rep_sb[:, 0:1], in1=inv_na[:], op=OP.mult
            )
            r_sb = keep.tile([NB, 1], F32)
            nc.scalar.activation(out=r_sb[:], in_=ratio[:], func=AF.Sqrt)

            cls_out = keep.tile([NB, C], F32)
            nc.vector.scalar_tensor_tensor(
                out=cls_out[:], in0=na_all[:], scalar=r_sb[:, 0:1],
                in1=cls_sb[:], op0=OP.mult, op1=OP.add,
            )
            nc.sync.dma_start(out=out_h[:, 0, :], in_=cls_out[:])

    nc.compile()
    return nc


def _topk_h2(nc, keep, mrg, gathp, psumt, catf, cvec_f, rowoff, ident,
             shifts, x_h, out_h):
    """Hierarchical top-256: per-chunk sort on 128 partitions, then a
    bitonic merge tree with (f32) index tracking.  Stage ops are split
    across DVE (compare / max / min / predicated moves) and ACT (index
    copies); partition-crossing folds are PE shift-matmuls (values and
    indices are normal f32, exact under x*1 + 0 accumulation)."""
    # ---- per-chunk descending sort (values + in-chunk positions) ----
    v0 = keep.tile([P, P], F32)
    i0u = keep.tile([P, P], U32)
    for k in range(P // 8):
        sl = slice(8 * k, 8 * k + 8)
        nc.vector.max(out=v0[:, sl], in_=catf[:])
        nc.vector.max_index(
            out=i0u[:, sl], in_max=v0[:, sl], in_values=catf[:]
        )
        if k != P // 8 - 1:
            nc.vector.match_replace(
                out=catf[:], in_to_replace=v0[:, sl], in_values=catf[:],
                imm_value=-1.6e9,
            )
    # global node index n = pos*8 + c (as exact f32; n = p*8 + c)
    i0f = keep.tile([P, P], F32)
    nc.vector.tensor_copy(out=i0f[:], in_=i0u[:])
    n0 = keep.tile([P, P], F32)
    nc.vector.tensor_scalar(
        out=n0[:], in0=i0f[:], scalar1=8.0, scalar2=None, op0=OP.mult,
    )
    nc.vector.tensor_scalar(
        out=n0[:], in0=n0[:], scalar1=cvec_f[:, 0:1], scalar2=None,
        op0=OP.add,
    )

    cp_eng = nc.scalar if _CPQ == "act" else nc.vector
    min_eng = nc.gpsimd if _MINQ == "pool" else nc.vector

    def fold(dst, src, parts, width, dt, src_cols=None, ptag="fold"):
        """[2p, w] -> [p, 2w]: partitions m and m+p land in row m.
        Lower half is an ACT copy; the partition-crossing upper half is a
        PE shift-matmul (out[m,:] = src[m+p,:]) -- ~1us latency vs the
        multi-us DMA+semaphore path."""
        sw = width if src_cols is None else src_cols
        cp_eng.copy(out=dst[0:parts, 0:width], in_=src[0:parts, 0:sw])
        ps = psumt.tile([parts, width], F32, tag=ptag, space="PSUM")
        nc.tensor.matmul(
            out=ps[:], lhsT=shifts[parts][:], rhs=src[0:2 * parts, 0:sw],
            start=True, stop=True,
        )
        cp_eng.copy(out=dst[0:parts, width:2 * width], in_=ps[:])

    def stage(vin, vout, iin, iout, mask, width, d, split=False,
              top_only=False, deep_min=False, ovr=None):
        """One compare-exchange stage on [*, width] tiles (descending)."""
        if split:
            n = width // 2
            if ovr is not None:
                va, vb, ia, ib = ovr
            else:
                va = _ap_sub(vin, 0, [[1, n]])
                vb = _ap_sub(vin, width - 1, [[-1, n]])
                ia = _ap_sub(iin, 0, [[1, n]])
                ib = _ap_sub(iin, width - 1, [[-1, n]])
            vlo = _ap_sub(vout, 0, [[1, n]])
            vhi = _ap_sub(vout, n, [[1, n]])
            ilo = _ap_sub(iout, 0, [[1, n]])
            ihi = _ap_sub(iout, n, [[1, n]])
            m = _ap_sub(mask, 0, [[1, n]])
        else:
            nb = width // (2 * d)
            va = _ap_sub(vin, 0, [[2 * d, nb], [1, d]])
            vb = _ap_sub(vin, d, [[2 * d, nb], [1, d]])
            ia = _ap_sub(iin, 0, [[2 * d, nb], [1, d]])
            ib = _ap_sub(iin, d, [[2 * d, nb], [1, d]])
            vlo = _ap_sub(vout, 0, [[2 * d, nb], [1, d]])
            vhi = _ap_sub(vout, d, [[2 * d, nb], [1, d]])
            ilo = _ap_sub(iout, 0, [[2 * d, nb], [1, d]])
            ihi = _ap_sub(iout, d, [[2 * d, nb], [1, d]])
            m = _ap_sub(mask, 0, [[2 * d, nb], [1, d]])
        # reversed-stride APs (split stages) stay on DVE -- the validated
        # path for negative strides
        cp = nc.vector.tensor_copy if split else (
            lambda out, in_: cp_eng.copy(out=out, in_=in_))
        nc.vector.tensor_tensor(out=m, in0=va, in1=vb, op=OP.is_gt)
        nc.vector.tensor_tensor(out=vlo, in0=va, in1=vb, op=OP.max)
        cp(out=ilo, in_=ib)
        nc.vector.copy_predicated(out=ilo, mask=m, data=ia)
        if not top_only:
            vmin = nc.vector if deep_min else min_eng
            vmin.tensor_tensor(out=vhi, in0=va, in1=vb, op=OP.min)
            cp(out=ihi, in_=ia)
            nc.vector.copy_predicated(out=ihi, mask=m, data=ib)

    # ---- L1: merge chunk pairs -> sorted 256 per (b, pair) ----
    va1 = mrg.tile([P // 2, 2 * P], F32)
    vb1 = mrg.tile([P // 2, 2 * P], F32)
    ia1 = mrg.tile([P // 2, 2 * P], F32)
    ib1 = mrg.tile([P // 2, 2 * P], F32)
    m1 = mrg.tile([P // 2, 2 * P], I32)
    fold(va1, v0, P // 2, P, F32)
    fold(ia1, n0, P // 2, P, F32, ptag="tx")
    cur_v, cur_i, alt_v, alt_i = va1, ia1, vb1, ib1
    stage(cur_v[:], alt_v[:], cur_i[:], alt_i[:], m1[:], 2 * P, P, split=True)
    cur_v, alt_v = alt_v, cur_v
    cur_i, alt_i = alt_i, cur_i
    d = P // 2
    while d >= 1:
        stage(cur_v[:], alt_v[:], cur_i[:], alt_i[:], m1[:], 2 * P, d)
        cur_v, alt_v = alt_v, cur_v
        cur_i, alt_i = alt_i, cur_i
        d //= 2

    def merge_levels(cv, ci_, av, ai, mm, keepw, deep_min=False,
                     half_only=False):
        d = keepw // 2
        first = True
        h = keepw // 2
        while d >= 1:
            if half_only and not first:
                stage(cv[:, h:], av[:, h:], ci_[:, h:], ai[:, h:], mm[:],
                      h, d, deep_min=deep_min)
            else:
                stage(cv[:], av[:], ci_[:], ai[:], mm[:], keepw, d,
                      deep_min=deep_min)
            cv, av = av, cv
            ci_, ai = ai, ci_
            d //= 2
            first = False
        return cv, ci_

    def merge_top(v_in, i_in, parts, keepw, mrgp, tagp, deep_min=False,
                  half_only=False):
        """[2*parts, keepw-lists] -> [parts, keepw] sorted top keepw.
        With half_only, only ranks [keepw/2, keepw) come out sorted (the
        top block stays bitonic) -- the final levels run on half width."""
        vs = mrgp.tile([parts, 2 * keepw], F32, tag=f"vs{tagp}")
        is_ = mrgp.tile([parts, 2 * keepw], F32, tag=f"is{tagp}")
        fold(vs, v_in, parts, keepw, F32)
        fold(is_, i_in, parts, keepw, F32, ptag="tx")
        vA = mrgp.tile([parts, keepw], F32, tag=f"vA{tagp}")
        iA = mrgp.tile([parts, keepw], F32, tag=f"iA{tagp}")
        vB = mrgp.tile([parts, keepw], F32, tag=f"vB{tagp}")
        iB = mrgp.tile([parts, keepw], F32, tag=f"iB{tagp}")
        mm = mrgp.tile([parts, 2 * keepw], I32, tag=f"mm{tagp}")
        # split stage keeping only the top half
        stage(vs[:], vA[:], is_[:], iA[:], mm[:], 2 * keepw, keepw,
              split=True, top_only=True)
        cv, ci_ = merge_levels(vA, iA, vB, iB, mm, keepw, deep_min=deep_min,
                               half_only=half_only)
        return cv, ci_, vs, is_

    # ---- L2: [64,256]x2 -> [32, 256] top halves ----
    v2, i2, _, _ = merge_top(cur_v, cur_i, P // 4, 2 * P, mrg, "a")

    def emit_out_half(idx_rev_src, lo):
        """Gather + write out rows [1+lo, 1+lo+128) from reversed-rank
        node-index AP idx_rev_src ([16, 128] f32, ascending out order)."""
        rows = keep.tile([NB, P], F32, tag=f"rows{lo}")
        nc.vector.tensor_copy(out=rows[:], in_=idx_rev_src)
        nc.vector.tensor_scalar(
            out=rows[:], in0=rows[:], scalar1=rowoff[:, 0:1], scalar2=None,
            op0=OP.add,
        )
        r_ps = psumt.tile([P, NB], F32, tag="tx", space="PSUM")
        nc.tensor.transpose(out=r_ps[:], in_=rows[:], identity=ident[:NB, :NB])
        rowst = keep.tile([P, NB], I32, tag=f"rowst{lo}")
        nc.vector.tensor_copy(out=rowst[:], in_=r_ps[:])
        for bb in range(NB):
            g = gathp.tile([P, C], F32, tag="g")
            nc.gpsimd.indirect_dma_start(
                out=g[:], out_offset=None, in_=x_h[:, :],
                in_offset=bass.IndirectOffsetOnAxis(
                    ap=rowst[:, bb:bb + 1], axis=0),
            )
            nc.sync.dma_start(out=out_h[bb, 1 + lo:1 + lo + P, :], in_=g[:])

    # ---- L3 folds: [32, 256]x2 -> [16, 512] value/index planes.  Both
    # the quick top-128 (Lq, prefixes at cols [0:128) and [256:384)) and
    # the full L3 merge read these. ----
    vs3 = mrg.tile([NB, 4 * P], F32, tag="vsb")
    is3 = mrg.tile([NB, 4 * P], F32, tag="isb")
    fold(vs3, v2, NB, 2 * P, F32)
    fold(is3, i2, NB, 2 * P, F32, ptag="tx")

    # ---- quick top-128 (ranks 0..127) from the two top-128 prefixes,
    # so the h=1 gathers overlap the full L3 merge ----
    vqA = mrg.tile([NB, P], F32, tag="vAq")
    iqA = mrg.tile([NB, P], F32, tag="iAq")
    vqB = mrg.tile([NB, P], F32, tag="vBq")
    iqB = mrg.tile([NB, P], F32, tag="iBq")
    # Lq and L3 share one mask tile: the WAR dependency on it forces the
    # whole L3 chain after Lq, so the h1 gathers start as early as possible
    mq = mrg.tile([NB, 2 * P], I32, tag="mmq")
    stage(None, vqA[:], None, iqA[:], mq[:], 2 * P, P, split=True,
          top_only=True,
          ovr=(_ap_sub(vs3[:], 0, [[1, P]]),
               _ap_sub(vs3[:], 2 * P + P - 1, [[-1, P]]),
               _ap_sub(is3[:], 0, [[1, P]]),
               _ap_sub(is3[:], 2 * P + P - 1, [[-1, P]])))
    vq, iq = merge_levels(vqA, iqA, vqB, iqB, mq, P, deep_min=True)
    # ranks 0..127: out rows 129..256 ascending = rank 127-t
    emit_out_half(iq[:, P - 1::-1], P)

    # ---- L3: -> [16, 256] global descending top-256 + node indices.
    # Only ranks 128..255 are consumed (h1 comes from Lq), so the final
    # levels sort just that block (half_only). ----
    vA3 = mrg.tile([NB, 2 * P], F32, tag="vAb")
    iA3 = mrg.tile([NB, 2 * P], F32, tag="iAb")
    vB3 = mrg.tile([NB, 2 * P], F32, tag="vBb")
    iB3 = mrg.tile([NB, 2 * P], F32, tag="iBb")
    m3 = mq
    stage(vs3[:], vA3[:], is3[:], iA3[:], m3[:], 4 * P, 2 * P, split=True,
          top_only=True)
    v3, i3 = merge_levels(vA3, iA3, vB3, iB3, m3, 2 * P, deep_min=True,
                          half_only=True)
    # ranks 128..255: out rows 1..128 ascending = rank 255-j
    emit_out_half(i3[:, 2 * P - 1:P - 1:-1], 0)


_CACHE = {}
_LOCK = threading.Lock()


def _get_program(pool):
    with _LOCK:
        if pool not in _CACHE:
            if pool:
                _CACHE[pool] = _build_pool1()
            else:
                raise NotImplementedError("pool=0 path not implemented")
        return _CACHE[pool]


def kernel(x, cls_token, edge_aggregation, pool):
    x = np.ascontiguousarray(np.asarray(x, dtype=np.float32))
    cls_token = np.ascontiguousarray(np.asarray(cls_token, dtype=np.float32))
    ea = np.ascontiguousarray(np.asarray(edge_aggregation, dtype=np.float32))
    pool_i = int(np.asarray(pool))

    nc = _get_program(bool(pool_i))

    in_maps = []
    for core in range(N_CORES):
        s = slice(core * NB, (core + 1) * NB)
        in_maps.append({
            "x": x[s].reshape(NB * (HW + 1), C),
            "cls": cls_token[s].reshape(NB, C),
            "ea": ea[s].reshape(NB * HW, C),
        })
    kw = {}
    if os.environ.get("GA_TRACE"):
        kw = {"trace": True}
    res = run_bass_kernel_spmd(nc, in_maps, core_ids=list(range(N_CORES)), **kw)
    global _LAST_RESULTS, _LAST_EXEC_NS
    _LAST_RESULTS = res.results
    _LAST_EXEC_NS = res.exec_time_ns
    out = np.concatenate([res.results[c]["out"] for c in range(N_CORES)], axis=0)
    return out.reshape(B, 1 + KEEP, C)


_LAST_RESULTS = None
_LAST_EXEC_NS = None


# revision 53
# speedup vs baseline: 1.0121x; 1.0055x over previous
"""Trainium2 Bass kernel for the GA block (topk_masking).

Reference semantics (B=128, HW=1024, C=384), pool=1:
    ea   = mean(edge_aggregation, axis=1)            # (B, 1, C)
    ci   = sigmoid(ea)                               # channel importance
    ca   = nodes @ ci                                # (B, HW) node scores
    ni   = sigmoid(ca)
    na   = ni @ nodes                                # (B, C)
    r    = ||cls||_F / ||na||_F   (global over the whole batch)
    cls' = cls + r * na
    out  = concat([cls', nodes sorted ascending by ca, top quarter kept])

Sharding: pure data parallel, 16 batches per core on 8 cores; the global
norms are combined with a tiny AllReduce of squared sums.

Numeric contract: the kept-node ordering must match the reference argsort
of its fp32 scores exactly, so the score computation keeps the verified
structure (pairwise ea fold -> 4 accumulating PE matmuls -> sigmoid;
per-chunk split-half sequential sums of fp32 products).  Scores are
compared in an int32 (x * 2^24) domain.

v3 engine-balanced layout: the cost model serializes DMA transfers on the
issuing queue (SP / ACT-hwdge / Pool-swdge run in parallel), so input
loads are spread across queues and interleaved with compute via a
4-stage software pipeline (load b | ci b-1 | scores b-2 | na b-3):
  SP    nodes DMA;  ACT  ea DMA + sigmoids;  Pool  fold + 6 products +
  na staging;  DVE  2 products + all 8 split-half reduces + misc;
  PE    ci fp32 matmuls + na float32r matmuls (1 cyc/row vs 4).
The topk tail splits each bitonic-merge stage across engines (DVE
compare/max/predicated moves, ACT index copies, Pool mins for L1/L2),
folds bounce on the SP queue, gathers stay on Pool (indirect DMA is
gpsimd-only), node-row writes go on SP.  The norm AllReduce occupies
only the collective cores, overlapping the whole merge tree.
"""

import math
import os
import threading

import numpy as np

import concourse.bass as bass
import concourse.tile as tile
from concourse import bacc, mybir
from concourse.bass_utils import run_bass_kernel_spmd
from concourse.masks import make_identity

F32 = mybir.dt.float32
F32R = mybir.dt.float32r
BF16 = mybir.dt.bfloat16
I32 = mybir.dt.int32
U32 = mybir.dt.uint32
AF = mybir.ActivationFunctionType
OP = mybir.AluOpType

N_CORES = 8
B = 128
HW = 1024
C = 384
NB = B // N_CORES          # batches per core
P = 128
NCH = HW // P              # 8 free-dim chunks of 128 node rows
KEEP = HW // 4             # 256

_PRODP = int(os.environ.get("GA_PRODP", "7"))   # product chunks on Pool
_CASTP = int(os.environ.get("GA_CASTP", "6"))   # bf16-cast chunks on Pool
_NA16 = int(os.environ.get("GA_NA16", "1"))     # 1: bf16 na matmuls
_MINQ = os.environ.get("GA_MINQ", "dve")        # merge min engine (Pool is
                                                # blocked by the collective)
_CPQ = os.environ.get("GA_CPQ", "act")          # stage index-copy engine


def _ap_sub(t_ap, off, pairs):
    """AP with the tile's partition dim plus explicit free-dim pattern."""
    return bass.AP(t_ap.tensor, t_ap.offset + off, [list(t_ap.ap[0])] + pairs)


def _f32r(ap):
    return ap.bitcast(F32R)


def _build_pool1():
    nc = bacc.Bacc(
        "TRN2",
        target_bir_lowering=False,
        debug=False,
        enable_asserts=False,
        num_devices=N_CORES,
    )
    x_h = nc.dram_tensor("x", [NB * (HW + 1), C], F32, kind="ExternalInput")
    cls_h = nc.dram_tensor("cls", [NB, C], F32, kind="ExternalInput")
    ea_h = nc.dram_tensor("ea", [NB * HW, C], F32, kind="ExternalInput")
    out_h = nc.dram_tensor("out", [NB, 1 + KEEP, C], F32, kind="ExternalOutput")

    cc_in = nc.dram_tensor("cc_in", [1, 2], F32)
    cc_out = nc.dram_tensor("cc_out", [N_CORES, 2], F32)

    with tile.TileContext(nc) as tc:
        with (
            tc.tile_pool(name="consts", bufs=1) as consts,
            tc.tile_pool(name="eap", bufs=3) as eap,
            tc.tile_pool(name="ndp", bufs=3) as ndp,
            tc.tile_pool(name="work", bufs=3) as work,
            tc.tile_pool(name="workq", bufs=2) as workq,
            tc.tile_pool(name="keep", bufs=1) as keep,
            tc.tile_pool(name="mrg", bufs=1) as mrg,
            tc.tile_pool(name="gath", bufs=8) as gathp,
            tc.tile_pool(name="psum", bufs=2, space="PSUM") as psum,
            tc.tile_pool(name="psumt", bufs=1, space="PSUM") as psumt,
            tc.tile_pool(name="psumn", bufs=1, space="PSUM") as psumn,
        ):
            ones128 = consts.tile([P, P], F32)
            nc.vector.memset(ones128[:], 1.0)
            ident = consts.tile([P, P], F32)
            make_identity(nc, ident[:])
            # row offset of node 0 of local batch b in the flattened x
            rowoff_i = consts.tile([NB, 1], I32)
            nc.gpsimd.iota(
                rowoff_i[:], pattern=[[0, 1]], base=1,
                channel_multiplier=HW + 1,
            )
            rowoff = consts.tile([NB, 1], F32)
            nc.vector.tensor_copy(out=rowoff[:], in_=rowoff_i[:])
            # partition q holds chunk c of batch b with q = 64*c0+32*c1+16*c2+b
            # (c = 4*c2+2*c1+c0), so each merge level folds contiguous
            # partition halves. cvec_f holds c(q) as an f32 per-partition
            # scalar for rebuilding node indices after the per-chunk sort.
            qvec = consts.tile([P, 1], I32)
            nc.gpsimd.iota(
                qvec[:], pattern=[[0, 1]], base=0, channel_multiplier=1,
            )
            cvec = consts.tile([P, 1], I32)
            scr1 = consts.tile([P, 1], I32)
            # c = 4*bit4(q) + 2*bit5(q) + bit6(q)
            nc.vector.tensor_scalar(
                out=cvec[:], in0=qvec[:], scalar1=4, scalar2=1,
                op0=OP.logical_shift_right, op1=OP.bitwise_and,
            )
            nc.vector.tensor_scalar(
                out=cvec[:], in0=cvec[:], scalar1=2, scalar2=None,
                op0=OP.logical_shift_left,
            )
            nc.vector.tensor_scalar(
                out=scr1[:], in0=qvec[:], scalar1=4, scalar2=2,
                op0=OP.logical_shift_right, op1=OP.bitwise_and,
            )
            nc.vector.tensor_tensor(
                out=cvec[:], in0=cvec[:], in1=scr1[:], op=OP.bitwise_or,
            )
            nc.vector.tensor_scalar(
                out=scr1[:], in0=qvec[:], scalar1=6, scalar2=1,
                op0=OP.logical_shift_right, op1=OP.bitwise_and,
            )
            nc.vector.tensor_tensor(
                out=cvec[:], in0=cvec[:], in1=scr1[:], op=OP.bitwise_or,
            )
            cvec_f = consts.tile([P, 1], F32)
            nc.vector.tensor_copy(out=cvec_f[:], in_=cvec[:])
            # partition-0 one-hot rows: segment l of [1, NB*NB] is e_l, used
            # as lhsT to land na rows in psum partition l (lhsT base
            # partition must be 0).
            onehot = consts.tile([1, NB * NB], F32)
            nc.vector.memset(onehot[:], 0.0)
            for l in range(NB):
                nc.vector.memset(onehot[0:1, l * (NB + 1):l * (NB + 1) + 1],
                                 1.0)
            # shift-by-p permutation matrices (shift[k, m] = 1 iff k == m+p):
            # lhsT of the PE "fold" matmuls that move partitions [p, 2p) of a
            # merge tile down to [0, p) without a (slow, sem-laggy) DMA.
            shifts = {}
            for pp in (P // 2, P // 4, P // 8):
                sh = consts.tile([2 * pp, pp], F32, tag=f"sh{pp}")
                rowk_i = consts.tile([2 * pp, 1], I32, tag=f"rki{pp}")
                nc.gpsimd.iota(rowk_i[:], pattern=[[0, 1]], base=0,
                               channel_multiplier=1)
                rowk = consts.tile([2 * pp, 1], F32, tag=f"rk{pp}")
                nc.vector.tensor_copy(out=rowk[:], in_=rowk_i[:])
                colj_i = consts.tile([2 * pp, pp], I32, tag=f"cji{pp}")
                nc.gpsimd.iota(colj_i[:], pattern=[[1, pp]], base=pp,
                               channel_multiplier=0)
                di = consts.tile([2 * pp, pp], F32, tag=f"di{pp}")
                nc.vector.tensor_copy(out=di[:], in_=colj_i[:])
                nc.vector.tensor_scalar(
                    out=di[:], in0=di[:], scalar1=rowk[:, 0:1],
                    scalar2=None, op0=OP.subtract,
                )
                nc.vector.tensor_scalar(
                    out=sh[:], in0=di[:], scalar1=0.0, scalar2=None,
                    op0=OP.is_equal,
                )
                shifts[pp] = sh

            # fp32 node scores, laid out [p, b*8+c]
            ca_allf = keep.tile([P, NB * NCH], F32)
            na_all = keep.tile([NB, C], F32)
            cls_sb = keep.tile([NB, C], F32)
            nc.sync.dma_start(out=cls_sb[:], in_=cls_h[:, :])

            def _emit_ci(b, ea_t, ci_rep):
                fold = work.tile([P, NCH // 2, C], F32, tag="fold")
                # pairwise fold (rows 2j + 2j+1): verified ci rounding draw
                nc.gpsimd.tensor_tensor(
                    out=fold[:], in0=ea_t[:, 0::2, :], in1=ea_t[:, 1::2, :],
                    op=OP.add,
                )
                psum_ci = psum.tile([P, C], F32, tag="psci", space="PSUM")
                for c in range(4):
                    nc.tensor.matmul(
                        out=psum_ci[:], lhsT=ones128[:], rhs=fold[:, c, :],
                        start=(c == 0), stop=(c == 3),
                    )
                nc.scalar.activation(
                    out=ci_rep[:], in_=psum_ci[:], func=AF.Sigmoid,
                    scale=1.0 / HW,
                )

            # batch 15's ea is loaded first (on the otherwise-idle Pool
            # queue) and its ci computed up front, so batch 15's scores
            # only trail its own nodes DMA
            ea15 = keep.tile([P, NCH, C], F32)
            nc.gpsimd.dma_start(
                out=ea15[:],
                in_=ea_h[(NB - 1) * HW:NB * HW, :].rearrange(
                    "(p c) f -> p c f", p=P),
            )
            ci15 = keep.tile([P, C], F32)
            _emit_ci(NB - 1, ea15, ci15[:])

            st = [dict() for _ in range(NB)]

            def stage_load(b):
                nt = ndp.tile([P, NCH, C], F32, tag="nd")
                nc.sync.dma_start(
                    out=nt[:],
                    in_=x_h[
                        b * (HW + 1) + 1:(b + 1) * (HW + 1), :
                    ].rearrange("(p c) f -> p c f", p=P),
                )
                st[b]["nodes"] = nt
                if b != NB - 1:
                    ea_t = eap.tile([P, NCH, C], F32, tag="ea")
                    nc.scalar.dma_start(
                        out=ea_t[:],
                        in_=ea_h[b * HW:(b + 1) * HW, :].rearrange(
                            "(p c) f -> p c f", p=P),
                    )
                    st[b]["ea"] = ea_t

            def stage_ci(j):
                if j == NB - 1:
                    st[j]["ci"] = ci15
                    return
                ci_rep = work.tile([P, C], F32, tag="ci")
                _emit_ci(j, st[j]["ea"], ci_rep[:])
                st[j]["ci"] = ci_rep

            def stage_score(k):
                nodes = st[k]["nodes"]
                ci_rep = st[k]["ci"]
                p_t = workq.tile([P, NCH, C], F32, tag="q")
                for c in range(NCH):
                    eng = nc.gpsimd if c < _PRODP else nc.vector
                    eng.tensor_tensor(
                        out=p_t[:, c, :],
                        in0=nodes[:, c, :],
                        in1=ci_rep[:], op=OP.mult,
                    )

                # per-(chunk, half) sequential fp32 sums: one reduce per
                # chunk collapsing the innermost 192-wide halves (same
                # per-half sequential order as the verified vred variant)
                ca_h = work.tile([P, NCH, 2], F32, tag="cah")
                for c in range(NCH):
                    eng = nc.vector
                    eng.tensor_reduce(
                        out=ca_h[:, c, :],
                        in_=_ap_sub(p_t[:, c, :], 0, [[C // 2, 2], [1, C // 2]]),
                        axis=mybir.AxisListType.X,
                        op=OP.add,
                    )
                ca_f = work.tile([P, NCH], F32, tag="caf")
                nc.vector.tensor_tensor(
                    out=ca_f[:], in0=ca_h[:, :, 0], in1=ca_h[:, :, 1],
                    op=OP.add,
                )
                # column q(b,c) = 64*c0 + 32*c1 + 16*c2 + b (bit-reversed c)
                nc.vector.tensor_copy(
                    out=_ap_sub(ca_allf[:], k, [[64, 2], [32, 2], [16, 2]]),
                    in_=_ap_sub(ca_f[:], 0, [[1, 2], [2, 2], [4, 2]]),
                )
                ni_t = work.tile([P, NCH], BF16 if _NA16 else F32, tag="ni")
                nc.scalar.activation(
                    out=ni_t[:], in_=ca_f[:], func=AF.Sigmoid,
                )
                st[k]["ni"] = ni_t

            # na_all rows are assembled in PSUM by one-hot float32r matmuls
            # (row l of psum_naall accumulates na_stage via lhsT=ident[l,:16])
            # -- avoids 16 lane-crossing Pool DMAs.
            psum_naall = psumn.tile([NB, C], F32, tag="naall", space="PSUM")

            def stage_na(l):
                ni_t = st[l]["ni"]
                if _NA16:
                    # bf16 shadow of the nodes for the (loose-tolerance) na
                    # matmuls, split across Pool and DVE.  For the last
                    # batches the cast rides the (by then idle) ACT queue so
                    # the topk-critical score drain isn't queued behind it.
                    nodes = workq.tile([P, NCH, C], BF16, tag="n16")
                    if l >= NB - 3:
                        nc.scalar.copy(out=nodes[:], in_=st[l]["nodes"][:])
                    else:
                        if _CASTP:
                            nc.gpsimd.tensor_copy(
                                out=nodes[:, :_CASTP, :],
                                in_=st[l]["nodes"][:, :_CASTP, :])
                        if _CASTP < NCH:
                            nc.vector.tensor_copy(
                                out=nodes[:, _CASTP:, :],
                                in_=st[l]["nodes"][:, _CASTP:, :])
                else:
                    nodes = st[l]["nodes"]
                psum_na = psum.tile([1, C], F32, tag="psna", space="PSUM")
                for c in range(NCH):
                    nc.tensor.matmul(
                        out=psum_na[:], lhsT=ni_t[:, c:c + 1],
                        rhs=nodes[:, c, :],
                        start=(c == 0), stop=(c == NCH - 1),
                    )
                na_stage = work.tile([1, C], F32, tag="nastage")
                nc.vector.tensor_copy(out=na_stage[:], in_=psum_na[:])
                nc.tensor.matmul(
                    out=psum_naall[:],
                    lhsT=onehot[0:1, l * NB:(l + 1) * NB],
                    rhs=na_stage[:],
                    start=(l == 0), stop=(l == NB - 1),
                    skip_group_check=True,
                )

            # 3-deep software pipeline: each engine's per-slot work only
            # depends on results from earlier slots (or earlier ops in the
            # same slot), so the per-engine in-order streams barely stall.
            for slot in range(NB + 2):
                if slot < NB:
                    stage_load(slot)
                if 0 <= slot - 1 < NB:
                    stage_ci(slot - 1)
                if 0 <= slot - 2 < NB:
                    stage_score(slot - 2)
                    stage_na(slot - 2)

            # ---- transpose scores into the per-chunk sort layout.  The
            # sort/merge compares raw f32 scores directly (exact compares;
            # strictly fewer key collisions than the old int32 x*2^24
            # domain, and normal floats survive the PE fold matmuls). ----
            t_ps = psumt.tile([P, P], F32, tag="tx", space="PSUM")
            nc.tensor.transpose(out=t_ps[:], in_=ca_allf[:], identity=ident[:])
            catf = keep.tile([P, P], F32)
            nc.scalar.copy(out=catf[:], in_=t_ps[:])

            # ---- global norm ratio: partial squared sums + AllReduce.
            # The collective blocks the Pool engine for its full modeled
            # duration, so it is launched as early as possible and Pool is
            # kept otherwise idle until the (post-collective) gathers. ----
            nc.vector.tensor_copy(out=na_all[:], in_=psum_naall[:])
            sq2 = keep.tile([NB, 2], F32)
            sq_scr = keep.tile([NB, C], F32)
            nc.scalar.activation(
                out=sq_scr[:], in_=cls_sb[:], func=AF.Square,
                accum_out=sq2[:, 0:1],
            )
            nc.scalar.activation(
                out=sq_scr[:], in_=na_all[:], func=AF.Square,
                accum_out=sq2[:, 1:2],
            )
            part_ps = psumt.tile([1, 2], F32, tag="cc2", space="PSUM")
            nc.tensor.matmul(
                out=part_ps[:], lhsT=ones128[:NB, 0:1], rhs=sq2[:],
                start=True, stop=True,
            )
            part_sb = keep.tile([1, 2], F32)
            nc.scalar.copy(out=part_sb[:], in_=part_ps[:])
            nc.sync.dma_start(out=cc_in[:], in_=part_sb[:])
            # AllGather + local sum instead of AllReduce: the collective
            # blocks the Pool engine for its modeled duration, and the cost
            # model prices AllGather at ~15us vs AllReduce's ~28us.
            nc.gpsimd.collective_compute(
                "AllGather",
                OP.bypass,
                replica_groups=[list(range(N_CORES))],
                ins=[cc_in[:].opt()],
                outs=[cc_out[:, :].opt()],
            )
            parts8 = keep.tile([1, 2, N_CORES], F32)
            nc.gpsimd.dma_start(
                out=parts8[:],
                in_=_ap_sub(cc_out[0:1, 0:1], 0, [[1, 2], [2, N_CORES]]))

            _topk_h2(nc, keep, mrg, gathp, psumt, catf, cvec_f, rowoff,
                     ident, shifts, x_h, out_h)

            # ---- post-collective cls row (emitted after the merge so no
            # engine stream stalls on the collective mid-topk) ----
            sums_sb = keep.tile([1, 2], F32)
            nc.vector.tensor_reduce(
                out=sums_sb[:], in_=parts8[:],
                axis=mybir.AxisListType.X, op=OP.add,
            )
            rep_ps = psumt.tile([NB, 2], F32, tag="cc2", space="PSUM")
            nc.tensor.matmul(
                out=rep_ps[:], lhsT=ones128[0:1, :NB], rhs=sums_sb[:],
                start=True, stop=True,
            )
            rep_sb = keep.tile([NB, 2], F32)
            nc.scalar.copy(out=rep_sb[:], in_=rep_ps[:])
            inv_na = keep.tile([NB, 1], F32)
            nc.vector.reciprocal(out=inv_na[:], in_=rep_sb[:, 1:2])
            ratio = keep.tile([NB, 1], F32)
            nc.vector.tensor_tensor(
                out=ratio[:], in0=rep_sb[:, 0:1], in1=inv_na[:], op=OP.mult
            )
            r_sb = keep.tile([NB, 1], F32)
            nc.scalar.activation(out=r_sb[:], in_=ratio[:], func=AF.Sqrt)

            cls_out = keep.tile([NB, C], F32)
            nc.vector.scalar_tensor_tensor(
                out=cls_out[:], in0=na_all[:], scalar=r_sb[:, 0:1],
                in1=cls_sb[:], op0=OP.mult, op1=OP.add,
            )
            nc.sync.dma_start(out=out_h[:, 0, :], in_=cls_out[:])

    nc.compile()
    return nc


def _topk_h2(nc, keep, mrg, gathp, psumt, catf, cvec_f, rowoff, ident,
             shifts, x_h, out_h):
    """Hierarchical top-256: per-chunk sort on 128 partitions, then a
    bitonic merge tree with (f32) index tracking.  Stage ops are split
    across DVE (compare / max / min / predicated moves) and ACT (index
    copies); partition-crossing folds are PE shift-matmuls (values and
    indices are normal f32, exact under x*1 + 0 accumulation)."""
    # ---- per-chunk descending sort (values + in-chunk positions) ----
    v0 = keep.tile([P, P], F32)
    i0u = keep.tile([P, P], U32)
    for k in range(P // 8):
        sl = slice(8 * k, 8 * k + 8)
        nc.vector.max(out=v0[:, sl], in_=catf[:])
        nc.vector.max_index(
            out=i0u[:, sl], in_max=v0[:, sl], in_values=catf[:]
        )
        if k != P // 8 - 1:
            nc.vector.match_replace(
                out=catf[:], in_to_replace=v0[:, sl], in_values=catf[:],
                imm_value=-1.6e9,
            )
    # global node index n = pos*8 + c (as exact f32; n = p*8 + c)
    i0f = keep.tile([P, P], F32)
    nc.vector.tensor_copy(out=i0f[:], in_=i0u[:])
    n0 = keep.tile([P, P], F32)
    nc.vector.tensor_scalar(
        out=n0[:], in0=i0f[:], scalar1=8.0, scalar2=None, op0=OP.mult,
    )
    nc.vector.tensor_scalar(
        out=n0[:], in0=n0[:], scalar1=cvec_f[:, 0:1], scalar2=None,
        op0=OP.add,
    )

    cp_eng = nc.scalar if _CPQ == "act" else nc.vector
    min_eng = nc.gpsimd if _MINQ == "pool" else nc.vector

    # Two mask tiles, alternated per stage: with a single mask every
    # stage's is_gt WARs against the previous stage's predicated copies,
    # serializing the whole chain at ~1.4us/stage.  Ping-ponging leaves
    # only a two-stage-spaced WAR.  Shared across all levels, which also
    # keeps the level order (notably Lq before L3, so the h1 gathers
    # start as early as possible).
    mka = mrg.tile([P // 2, 2 * P], I32, tag="mka")
    mkb = mrg.tile([P // 2, 2 * P], I32, tag="mkb")
    mks = [mka, mkb]
    mtog = [0]

    def next_mask(parts):
        mk = mks[mtog[0]]
        mtog[0] ^= 1
        return mk[0:parts]

    def fold(dst, src, parts, width, dt, src_cols=None, ptag="fold"):
        """[2p, w] -> [p, 2w]: partitions m and m+p land in row m.
        Lower half is an ACT copy; the partition-crossing upper half is a
        PE shift-matmul (out[m,:] = src[m+p,:]) -- ~1us latency vs the
        multi-us DMA+semaphore path."""
        sw = width if src_cols is None else src_cols
        cp_eng.copy(out=dst[0:parts, 0:width], in_=src[0:parts, 0:sw])
        ps = psumt.tile([parts, width], F32, tag=ptag, space="PSUM")
        nc.tensor.matmul(
            out=ps[:], lhsT=shifts[parts][:], rhs=src[0:2 * parts, 0:sw],
            start=True, stop=True,
        )
        cp_eng.copy(out=dst[0:parts, width:2 * width], in_=ps[:])

    def stage(vin, vout, iin, iout, width, d, split=False,
              top_only=False, deep_min=False, ovr=None):
        """One compare-exchange stage on [*, width] tiles (descending)."""
        mask = next_mask(vout.ap[0][1])
        if split:
            n = width // 2
            if ovr is not None:
                va, vb, ia, ib = ovr
            else:
                va = _ap_sub(vin, 0, [[1, n]])
                vb = _ap_sub(vin, width - 1, [[-1, n]])
                ia = _ap_sub(iin, 0, [[1, n]])
                ib = _ap_sub(iin, width - 1, [[-1, n]])
            vlo = _ap_sub(vout, 0, [[1, n]])
            vhi = _ap_sub(vout, n, [[1, n]])
            ilo = _ap_sub(iout, 0, [[1, n]])
            ihi = _ap_sub(iout, n, [[1, n]])
            m = _ap_sub(mask, 0, [[1, n]])
        else:
            nb = width // (2 * d)
            va = _ap_sub(vin, 0, [[2 * d, nb], [1, d]])
            vb = _ap_sub(vin, d, [[2 * d, nb], [1, d]])
            ia = _ap_sub(iin, 0, [[2 * d, nb], [1, d]])
            ib = _ap_sub(iin, d, [[2 * d, nb], [1, d]])
            vlo = _ap_sub(vout, 0, [[2 * d, nb], [1, d]])
            vhi = _ap_sub(vout, d, [[2 * d, nb], [1, d]])
            ilo = _ap_sub(iout, 0, [[2 * d, nb], [1, d]])
            ihi = _ap_sub(iout, d, [[2 * d, nb], [1, d]])
            m = _ap_sub(mask, 0, [[2 * d, nb], [1, d]])
        # reversed-stride APs (split stages) stay on DVE -- the validated
        # path for negative strides
        cp = nc.vector.tensor_copy if split else (
            lambda out, in_: cp_eng.copy(out=out, in_=in_))
        nc.vector.tensor_tensor(out=m, in0=va, in1=vb, op=OP.is_gt)
        nc.vector.tensor_tensor(out=vlo, in0=va, in1=vb, op=OP.max)
        cp(out=ilo, in_=ib)
        nc.vector.copy_predicated(out=ilo, mask=m, data=ia)
        if not top_only:
            vmin = nc.vector if deep_min else min_eng
            vmin.tensor_tensor(out=vhi, in0=va, in1=vb, op=OP.min)
            cp(out=ihi, in_=ia)
            nc.vector.copy_predicated(out=ihi, mask=m, data=ib)

    # ---- L1: merge chunk pairs -> sorted 256 per (b, pair) ----
    va1 = mrg.tile([P // 2, 2 * P], F32)
    vb1 = mrg.tile([P // 2, 2 * P], F32)
    ia1 = mrg.tile([P // 2, 2 * P], F32)
    ib1 = mrg.tile([P // 2, 2 * P], F32)
    fold(va1, v0, P // 2, P, F32)
    fold(ia1, n0, P // 2, P, F32, ptag="tx")
    cur_v, cur_i, alt_v, alt_i = va1, ia1, vb1, ib1
    stage(cur_v[:], alt_v[:], cur_i[:], alt_i[:], 2 * P, P, split=True)
    cur_v, alt_v = alt_v, cur_v
    cur_i, alt_i = alt_i, cur_i
    d = P // 2
    while d >= 1:
        stage(cur_v[:], alt_v[:], cur_i[:], alt_i[:], 2 * P, d)
        cur_v, alt_v = alt_v, cur_v
        cur_i, alt_i = alt_i, cur_i
        d //= 2

    def merge_levels(cv, ci_, av, ai, keepw, deep_min=False,
                     half_only=False):
        d = keepw // 2
        first = True
        h = keepw // 2
        while d >= 1:
            if half_only and not first:
                stage(cv[:, h:], av[:, h:], ci_[:, h:], ai[:, h:],
                      h, d, deep_min=deep_min)
            else:
                stage(cv[:], av[:], ci_[:], ai[:], keepw, d,
                      deep_min=deep_min)
            cv, av = av, cv
            ci_, ai = ai, ci_
            d //= 2
            first = False
        return cv, ci_

    def merge_top(v_in, i_in, parts, keepw, mrgp, tagp, deep_min=False,
                  half_only=False):
        """[2*parts, keepw-lists] -> [parts, keepw] sorted top keepw.
        With half_only, only ranks [keepw/2, keepw) come out sorted (the
        top block stays bitonic) -- the final levels run on half width."""
        vs = mrgp.tile([parts, 2 * keepw], F32, tag=f"vs{tagp}")
        is_ = mrgp.tile([parts, 2 * keepw], F32, tag=f"is{tagp}")
        fold(vs, v_in, parts, keepw, F32)
        fold(is_, i_in, parts, keepw, F32, ptag="tx")
        vA = mrgp.tile([parts, keepw], F32, tag=f"vA{tagp}")
        iA = mrgp.tile([parts, keepw], F32, tag=f"iA{tagp}")
        vB = mrgp.tile([parts, keepw], F32, tag=f"vB{tagp}")
        iB = mrgp.tile([parts, keepw], F32, tag=f"iB{tagp}")
        # split stage keeping only the top half
        stage(vs[:], vA[:], is_[:], iA[:], 2 * keepw, keepw,
              split=True, top_only=True)
        cv, ci_ = merge_levels(vA, iA, vB, iB, keepw, deep_min=deep_min,
                               half_only=half_only)
        return cv, ci_, vs, is_

    # ---- L2: [64,256]x2 -> [32, 256] top halves ----
    v2, i2, _, _ = merge_top(cur_v, cur_i, P // 4, 2 * P, mrg, "a")

    def emit_out_half(idx_rev_src, lo):
        """Gather + write out rows [1+lo, 1+lo+128) from reversed-rank
        node-index AP idx_rev_src ([16, 128] f32, ascending out order)."""
        rows = keep.tile([NB, P], F32, tag=f"rows{lo}")
        nc.vector.tensor_copy(out=rows[:], in_=idx_rev_src)
        nc.vector.tensor_scalar(
            out=rows[:], in0=rows[:], scalar1=rowoff[:, 0:1], scalar2=None,
            op0=OP.add,
        )
        r_ps = psumt.tile([P, NB], F32, tag="tx", space="PSUM")
        nc.tensor.transpose(out=r_ps[:], in_=rows[:], identity=ident[:NB, :NB])
        rowst = keep.tile([P, NB], I32, tag=f"rowst{lo}")
        nc.vector.tensor_copy(out=rowst[:], in_=r_ps[:])
        for bb in range(NB):
            g = gathp.tile([P, C], F32, tag="g")
            nc.gpsimd.indirect_dma_start(
                out=g[:], out_offset=None, in_=x_h[:, :],
                in_offset=bass.IndirectOffsetOnAxis(
                    ap=rowst[:, bb:bb + 1], axis=0),
            )
            nc.sync.dma_start(out=out_h[bb, 1 + lo:1 + lo + P, :], in_=g[:])

    # ---- L3 folds: [32, 256]x2 -> [16, 512] value/index planes.  Both
    # the quick top-128 (Lq, prefixes at cols [0:128) and [256:384)) and
    # the full L3 merge read these. ----
    vs3 = mrg.tile([NB, 4 * P], F32, tag="vsb")
    is3 = mrg.tile([NB, 4 * P], F32, tag="isb")
    fold(vs3, v2, NB, 2 * P, F32)
    fold(is3, i2, NB, 2 * P, F32, ptag="tx")

    # ---- quick top-128 (ranks 0..127) from the two top-128 prefixes,
    # so the h=1 gathers overlap the full L3 merge ----
    vqA = mrg.tile([NB, P], F32, tag="vAq")
    iqA = mrg.tile([NB, P], F32, tag="iAq")
    vqB = mrg.tile([NB, P], F32, tag="vBq")
    iqB = mrg.tile([NB, P], F32, tag="iBq")
    stage(None, vqA[:], None, iqA[:], 2 * P, P, split=True,
          top_only=True,
          ovr=(_ap_sub(vs3[:], 0, [[1, P]]),
               _ap_sub(vs3[:], 2 * P + P - 1, [[-1, P]]),
               _ap_sub(is3[:], 0, [[1, P]]),
               _ap_sub(is3[:], 2 * P + P - 1, [[-1, P]])))
    vq, iq = merge_levels(vqA, iqA, vqB, iqB, P, deep_min=True)
    # ranks 0..127: out rows 129..256 ascending = rank 127-t
    emit_out_half(iq[:, P - 1::-1], P)

    # ---- L3: -> [16, 256] global descending top-256 + node indices.
    # Only ranks 128..255 are consumed (h1 comes from Lq), so the final
    # levels sort just that block (half_only). ----
    vA3 = mrg.tile([NB, 2 * P], F32, tag="vAb")
    iA3 = mrg.tile([NB, 2 * P], F32, tag="iAb")
    vB3 = mrg.tile([NB, 2 * P], F32, tag="vBb")
    iB3 = mrg.tile([NB, 2 * P], F32, tag="iBb")
    stage(vs3[:], vA3[:], is3[:], iA3[:], 4 * P, 2 * P, split=True,
          top_only=True)
    v3, i3 = merge_levels(vA3, iA3, vB3, iB3, 2 * P, deep_min=True,
                          half_only=True)
    # ranks 128..255: out rows 1..128 ascending = rank 255-j
    emit_out_half(i3[:, 2 * P - 1:P - 1:-1], 0)


_CACHE = {}
_LOCK = threading.Lock()


def _get_program(pool):
    with _LOCK:
        if pool not in _CACHE:
            if pool:
                _CACHE[pool] = _build_pool1()
            else:
                raise NotImplementedError("pool=0 path not implemented")
        return _CACHE[pool]


def kernel(x, cls_token, edge_aggregation, pool):
    x = np.ascontiguousarray(np.asarray(x, dtype=np.float32))
    cls_token = np.ascontiguousarray(np.asarray(cls_token, dtype=np.float32))
    ea = np.ascontiguousarray(np.asarray(edge_aggregation, dtype=np.float32))
    pool_i = int(np.asarray(pool))

    nc = _get_program(bool(pool_i))

    in_maps = []
    for core in range(N_CORES):
        s = slice(core * NB, (core + 1) * NB)
        in_maps.append({
            "x": x[s].reshape(NB * (HW + 1), C),
            "cls": cls_token[s].reshape(NB, C),
            "ea": ea[s].reshape(NB * HW, C),
        })
    kw = {}
    if os.environ.get("GA_TRACE"):
        kw = {"trace": True}
    res = run_bass_kernel_spmd(nc, in_maps, core_ids=list(range(N_CORES)), **kw)
    global _LAST_RESULTS, _LAST_EXEC_NS
    _LAST_RESULTS = res.results
    _LAST_EXEC_NS = res.exec_time_ns
    out = np.concatenate([res.results[c]["out"] for c in range(N_CORES)], axis=0)
    return out.reshape(B, 1 + KEEP, C)


_LAST_RESULTS = None
_LAST_EXEC_NS = None
